# revision 2
# baseline (speedup 1.0000x reference)
"""Trainium2 Bass kernel for hyperbolic GNN aggregation (HGCN-style):

    out = proj(expmap0(mobius_matvec(adj, logmap0(x, c), c), c), c)

with x [8192, 64] fp32, adj [8192, 8192] fp32, c [1] fp32.

Strategy (8 NeuronCores, pure data parallel, no collectives):
  - Row-shard adj: core i owns output rows [1024*i, 1024*(i+1)).
  - Host feeds each core adj[rows, :].T (contiguous [8192, 1024]) so the
    PE contraction runs over the partition axis with no on-device
    transpose of the big matrix. For the default "split3" mode the shard
    is split into bf16 hi/lo planes (adj ~= hi + lo exactly captures
    ~16 mantissa bits); the device computes
        adj @ xt ~= hi@xt_hi + hi@xt_lo + lo@xt_hi
    in fp32 PSUM, giving ~1e-6 relative error at bf16 matmul speed
    (fp32 matmuls run at 1/4 rate on TRN2's PE).
  - x is replicated; each core computes logmap0(x) for all rows (cheap:
    all transcendentals act on row norms, [8192] values packed as one
    [128, 64] tile). Per-node post-matmul ops are local to the core.
  - mx arrives in PSUM transposed ([64, 1024]); PE identity-transposes
    it back to row-major [128, 8*64] for the row-norm chain.
  - artanh/tanh/rsqrt are built from Ln/Exp (one ACT table set) plus a
    Newton step on DVE; ACT Rsqrt/Reciprocal are banned (inaccurate) and
    Sqrt has a loose precision budget.

The kernel is compiled once per (mode, sqrt(c)) and cached.
"""

import numpy as np
import ml_dtypes

from concourse import bass, mybir, tile, bacc, masks
from concourse.bass_utils import run_bass_kernel_spmd

F32 = mybir.dt.float32
BF16 = mybir.dt.bfloat16
AF = mybir.ActivationFunctionType
OP = mybir.AluOpType

N, D, NC = 8192, 64, 8
ROWS = N // NC          # 1024 output rows per core
A = N // 128            # 64 row-groups of the replicated x
T = ROWS // 128         # 8 local row tiles
K = N // 128            # 64 contraction chunks

MIN_NORM_SQ = 1e-30     # clamp on squared norms == clamp(norm, 1e-15)
ATANH_EPS = 1e-7
BALL_EPS = 1e-5

MODE = "split3"         # "split3" | "fp32" | "bf16"

_BUILD_CACHE: dict = {}
LAST_PERF = None


def _bcast(ap, inner):
    """Append a zero-stride inner dim of size `inner` to an AP (free-dim
    broadcast of a per-(partition, group) scalar)."""
    return bass.AP(ap.tensor, ap.offset, list(ap.ap) + [[0, inner]])


def _view3(ap, d=D):
    return ap.rearrange("p (a d) -> p a d", d=d)


class _Emitter:
    """Helpers that emit the recurring DVE/ACT op patterns."""

    def __init__(self, nc, pool):
        self.nc = nc
        self.pool = pool
        self.n = 0

    def tmp(self, shape, dtype=F32):
        self.n += 1
        return self.pool.tile(shape, dtype, name=f"tmp{self.n}", tag=f"tmp{self.n}")

    def rsqrt(self, dst, ss):
        """dst = 1/sqrt(ss); ss pre-clamped > 0. Ln/Exp + one Newton step."""
        nc = self.nc
        t = self.tmp([128, ss.shape[1]])
        nc.scalar.activation(t[:], ss, AF.Ln)
        nc.scalar.activation(dst, t[:], AF.Exp, scale=-0.5)
        # r = r0 * (1.5 - 0.5 * ss * r0^2)
        nc.vector.tensor_mul(t[:], dst, dst)
        nc.vector.tensor_mul(t[:], t[:], ss)
        nc.vector.tensor_scalar(t[:], t[:], -0.5, 1.5, OP.mult, OP.add)
        nc.vector.tensor_mul(dst, dst, t[:])

    def artanh2(self, dst, z):
        """dst = 2 * artanh(z) = ln(1+z) - ln(1-z); z in [0, 1)."""
        nc = self.nc
        lp = self.tmp([128, z.shape[1]])
        nc.scalar.activation(lp[:], z, AF.Ln, bias=1.0, scale=1.0)
        nc.scalar.activation(dst, z, AF.Ln, bias=1.0, scale=-1.0)
        nc.vector.tensor_sub(dst, lp[:], dst)

    def tanh_of_half(self, dst, x2, scale=1.0):
        """dst = tanh(scale * x2 / 2) = 1 - 2/(exp(scale*x2) + 1)."""
        nc = self.nc
        nc.scalar.activation(dst, x2, AF.Exp, scale=scale)
        nc.vector.tensor_scalar_add(dst, dst, 1.0)
        nc.vector.reciprocal(dst, dst)
        nc.vector.tensor_scalar(dst, dst, -2.0, 1.0, OP.mult, OP.add)

    def sumsq_groups(self, dst, src, scratch, d=D):
        """dst[p, g] = sum_d src[p, g*d:(g+1)*d]^2."""
        nc = self.nc
        nc.vector.tensor_mul(scratch, src, src)
        nc.vector.reduce_sum(dst, _view3(scratch, d), axis=mybir.AxisListType.X)


def _build(mode: str, sc: float):
    """Trace + schedule the per-core SPMD program. Returns a finalized Bacc."""
    nc = bacc.Bacc("TRN2", target_bir_lowering=False, debug=False, num_devices=NC)

    xf_d = nc.dram_tensor("xf", [128, A * D], F32, kind="ExternalInput")
    xl_d = nc.dram_tensor("xl", [128, T * D], F32, kind="ExternalInput")
    if mode == "split3":
        ah_d = nc.dram_tensor("ah", [N, ROWS], BF16, kind="ExternalInput")
        al_d = nc.dram_tensor("al", [N, ROWS], BF16, kind="ExternalInput")
    elif mode == "bf16":
        ah_d = nc.dram_tensor("ah", [N, ROWS], BF16, kind="ExternalInput")
        al_d = None
    else:
        ah_d = nc.dram_tensor("ah", [N, ROWS], F32, kind="ExternalInput")
        al_d = None
    out_d = nc.dram_tensor("out", [128, T * D], F32, kind="ExternalOutput")

    mm_dt = F32 if mode == "fp32" else BF16

    with tile.TileContext(nc) as tc:
        with (
            tc.tile_pool(name="big", bufs=1) as big,
            tc.tile_pool(name="bchunks", bufs=8) as bpool,
            tc.tile_pool(name="small", bufs=1) as sm,
            tc.tile_pool(name="psum", bufs=1, space="PSUM") as pp,
        ):
            em = _Emitter(nc, sm)

            # ---------------- Phase A: xt = logmap0(x) for all N rows ----
            X = big.tile([128, A * D], F32)
            nc.sync.dma_start(X[:], xf_d.ap()[:])

            SQ = big.tile([128, A * D], F32)
            ss = sm.tile([128, A], F32)
            em.sumsq_groups(ss[:], X[:], SQ[:])
            nc.vector.tensor_scalar_max(ss[:], ss[:], MIN_NORM_SQ)
            r = sm.tile([128, A], F32)        # 1/xn
            em.rsqrt(r[:], ss[:])
            xn = sm.tile([128, A], F32)       # clamp(|x_i|, 1e-15)
            nc.vector.tensor_mul(xn[:], ss[:], r[:])
            z = sm.tile([128, A], F32)        # clip(sc*xn, <1)
            nc.vector.tensor_scalar(z[:], xn[:], sc, 1.0 - ATANH_EPS, OP.mult, OP.min)
            u2 = sm.tile([128, A], F32)       # 2*artanh(z)
            em.artanh2(u2[:], z[:])
            f = sm.tile([128, A], F32)        # artanh(z)/(sc*xn)
            nc.vector.tensor_mul(f[:], u2[:], r[:])
            nc.vector.tensor_scalar_mul(f[:], f[:], 0.5 / sc)

            XT = big.tile([128, A * D], F32)  # xt = f (.) x
            nc.vector.tensor_mul(_view3(XT[:]), _view3(X[:]), _bcast(f[:], D))

            if mode == "fp32":
                XH, XL = XT, None
            else:
                XH = big.tile([128, A * D], BF16)
                nc.vector.tensor_copy(XH[:], XT[:])
                if mode == "split3":
                    XL = big.tile([128, A * D], BF16)
                    nc.vector.tensor_sub(XL[:], XT[:], XH[:])
                else:
                    XL = None

            # ---------------- Matmul: mx.T = (adj_shard @ xt).T ----------
            ps0 = pp.tile([64, 512], F32)
            ps1 = pp.tile([64, 512], F32)
            for k in range(K):
                rows = slice(k * 128, (k + 1) * 128)
                ah_t = bpool.tile([128, ROWS], mm_dt, tag="ah")
                nc.sync.dma_start(ah_t[:], ah_d.ap()[rows, :])
                if mode == "split3":
                    al_t = bpool.tile([128, ROWS], BF16, tag="al")
                    nc.sync.dma_start(al_t[:], al_d.ap()[rows, :])
                xh_k = XH[:, k * D:(k + 1) * D]
                s, e = (k == 0), (k == K - 1)
                if mode == "split3":
                    xl_k = XL[:, k * D:(k + 1) * D]
                    nc.tensor.matmul(ps0[:], xl_k, ah_t[:, :512], start=s, stop=False)
                    nc.tensor.matmul(ps1[:], xl_k, ah_t[:, 512:], start=s, stop=False)
                    nc.tensor.matmul(ps0[:], xh_k, ah_t[:, :512], start=False, stop=False)
                    nc.tensor.matmul(ps1[:], xh_k, ah_t[:, 512:], start=False, stop=False)
                    nc.tensor.matmul(ps0[:], xh_k, al_t[:, :512], start=False, stop=e)
                    nc.tensor.matmul(ps1[:], xh_k, al_t[:, 512:], start=False, stop=e)
                else:
                    nc.tensor.matmul(ps0[:], xh_k, ah_t[:, :512], start=s, stop=e)
                    nc.tensor.matmul(ps1[:], xh_k, ah_t[:, 512:], start=s, stop=e)

            # ---------------- Transpose mx.T -> row-major ----------------
            mxT = sm.tile([64, ROWS], F32)
            nc.any.tensor_copy(mxT[:, :512], ps0[:])
            nc.any.tensor_copy(mxT[:, 512:], ps1[:])
            ident = sm.tile([128, 128], F32)
            masks.make_identity(nc, ident[:])
            psT = pp.tile([128, T * D], F32)
            for t in range(T):
                nc.tensor.transpose(
                    psT[:, t * D:(t + 1) * D],
                    mxT[:, t * 128:(t + 1) * 128],
                    ident[:64, :64],
                )
            MX = sm.tile([128, T * D], F32)
            nc.any.tensor_copy(MX[:], psT[:])

            # ------- Local ||xt|| for this core's rows (from x rows) -----
            XLo = sm.tile([128, T * D], F32)
            nc.sync.dma_start(XLo[:], xl_d.ap()[:])
            SQ2 = sm.tile([128, T * D], F32)
            ssl = sm.tile([128, T], F32)
            em.sumsq_groups(ssl[:], XLo[:], SQ2[:])
            nc.vector.tensor_scalar_max(ssl[:], ssl[:], MIN_NORM_SQ)
            rl = sm.tile([128, T], F32)
            em.rsqrt(rl[:], ssl[:])
            xnl = sm.tile([128, T], F32)
            nc.vector.tensor_mul(xnl[:], ssl[:], rl[:])
            zl = sm.tile([128, T], F32)
            nc.vector.tensor_scalar(zl[:], xnl[:], sc, 1.0 - ATANH_EPS, OP.mult, OP.min)
            u2l = sm.tile([128, T], F32)
            em.artanh2(u2l[:], zl[:])
            # xn_mob = clamp(||xt_row|| = artanh(z)/sc, 1e-15)
            xnm = sm.tile([128, T], F32)
            nc.vector.tensor_scalar(xnm[:], u2l[:], 0.5 / sc, 1e-15, OP.mult, OP.max)
            rxn = sm.tile([128, T], F32)
            nc.vector.reciprocal(rxn[:], xnm[:])
            z2 = sm.tile([128, T], F32)
            nc.vector.tensor_scalar(z2[:], xnm[:], sc, 1.0 - ATANH_EPS, OP.mult, OP.min)
            u22 = sm.tile([128, T], F32)      # 2*artanh(sc*xn_mob)
            em.artanh2(u22[:], z2[:])

            # ---------------- mobius scale: res = tanh(g)*mx/(mxn*sc) ----
            ssm = sm.tile([128, T], F32)
            em.sumsq_groups(ssm[:], MX[:], SQ2[:])
            nc.vector.tensor_scalar_max(ssm[:], ssm[:], MIN_NORM_SQ)
            rm = sm.tile([128, T], F32)       # 1/mxn
            em.rsqrt(rm[:], ssm[:])
            mxn = sm.tile([128, T], F32)
            nc.vector.tensor_mul(mxn[:], ssm[:], rm[:])
            g2 = sm.tile([128, T], F32)       # 2*g = mxn/xn * 2*artanh(sc*xn)
            nc.vector.tensor_mul(g2[:], mxn[:], rxn[:])
            nc.vector.tensor_mul(g2[:], g2[:], u22[:])
            tg = sm.tile([128, T], F32)       # tanh(g)
            em.tanh_of_half(tg[:], g2[:])
            s1 = sm.tile([128, T], F32)
            nc.vector.tensor_mul(s1[:], tg[:], rm[:])
            nc.vector.tensor_scalar_mul(s1[:], s1[:], 1.0 / sc)
            RES = sm.tile([128, T * D], F32)
            nc.vector.tensor_mul(_view3(RES[:]), _view3(MX[:]), _bcast(s1[:], D))

            # ---------------- expmap0 ------------------------------------
            ssr = sm.tile([128, T], F32)
            em.sumsq_groups(ssr[:], RES[:], SQ2[:])
            nc.vector.tensor_scalar_max(ssr[:], ssr[:], MIN_NORM_SQ)
            rr = sm.tile([128, T], F32)       # 1/un
            em.rsqrt(rr[:], ssr[:])
            un = sm.tile([128, T], F32)
            nc.vector.tensor_mul(un[:], ssr[:], rr[:])
            tw = sm.tile([128, T], F32)       # tanh(sc*un)
            em.tanh_of_half(tw[:], un[:], scale=2.0 * sc)
            s2 = sm.tile([128, T], F32)
            nc.vector.tensor_mul(s2[:], tw[:], rr[:])
            nc.vector.tensor_scalar_mul(s2[:], s2[:], 1.0 / sc)
            OUT = sm.tile([128, T * D], F32)
            nc.vector.tensor_mul(_view3(OUT[:]), _view3(RES[:]), _bcast(s2[:], D))

            # ---------------- proj ---------------------------------------
            sso = sm.tile([128, T], F32)
            em.sumsq_groups(sso[:], OUT[:], SQ2[:])
            nc.vector.tensor_scalar_max(sso[:], sso[:], MIN_NORM_SQ)
            ro = sm.tile([128, T], F32)
            em.rsqrt(ro[:], sso[:])
            fac = sm.tile([128, T], F32)      # min(maxnorm/|out|, 1)
            nc.vector.tensor_scalar(
                fac[:], ro[:], (1.0 - BALL_EPS) / sc, 1.0, OP.mult, OP.min
            )
            FIN = sm.tile([128, T * D], F32)
            nc.vector.tensor_mul(_view3(FIN[:]), _view3(OUT[:]), _bcast(fac[:], D))

            nc.sync.dma_start(out_d.ap()[:], FIN[:])

    nc.finalize()
    return nc


def _get_program(mode: str, sc: float):
    key = (mode, sc)
    if key not in _BUILD_CACHE:
        _BUILD_CACHE[key] = _build(mode, sc)
    return _BUILD_CACHE[key]


def _prep_x_tiles(xr: np.ndarray):
    """[G*128, D] row-major -> [128, G*D] with row g*128+p at [p, g*D:(g+1)*D]."""
    g = xr.shape[0] // 128
    return np.ascontiguousarray(
        xr.reshape(g, 128, D).transpose(1, 0, 2).reshape(128, g * D)
    )


def kernel(x: np.ndarray, adj: np.ndarray, c: np.ndarray,
           _trace: bool = False, _mode: str = None) -> np.ndarray:
    global LAST_PERF
    mode = _mode or MODE
    x = np.ascontiguousarray(np.asarray(x, dtype=np.float32))
    adj = np.ascontiguousarray(np.asarray(adj, dtype=np.float32))
    c32 = np.float32(np.asarray(c).reshape(-1)[0])
    sc = float(np.sqrt(c32))

    nc = _get_program(mode, sc)

    xf_arr = _prep_x_tiles(x)
    in_maps = []
    for i in range(NC):
        rows = slice(i * ROWS, (i + 1) * ROWS)
        bt = np.ascontiguousarray(adj[rows].T)          # [N, ROWS] fp32
        m = {"xf": xf_arr, "xl": _prep_x_tiles(x[rows])}
        if mode == "fp32":
            m["ah"] = bt
        elif mode == "bf16":
            m["ah"] = bt.astype(ml_dtypes.bfloat16)
        else:
            hi = bt.astype(ml_dtypes.bfloat16)
            m["ah"] = hi
            m["al"] = (bt - hi.astype(np.float32)).astype(ml_dtypes.bfloat16)
        in_maps.append(m)

    kwargs = {}
    if _trace:
        import profile_shim
        profile_shim.install()
        kwargs = {"trace": True}
    res = run_bass_kernel_spmd(nc, in_maps, core_ids=list(range(NC)), **kwargs)
    LAST_PERF = res

    outs = []
    for i in range(NC):
        o = res.results[i]["out"]                        # [128, T*D]
        outs.append(o.reshape(128, T, D).transpose(1, 0, 2).reshape(ROWS, D))
    return np.ascontiguousarray(np.concatenate(outs, axis=0), dtype=np.float32)
